# revision 1
# baseline (speedup 1.0000x reference)
"""Trainium2 Bass kernel for CosineAttention:

    out = sigmoid((xn @ xn.T) @ x)   where xn = x / ||x_row||

Key algebraic optimization: reassociate (xn @ xn.T) @ x = xn @ (xn.T @ x).
G = xn.T @ x is [D, D] — the O(N^2 D) similarity matrix is never formed.
Total work drops from ~275 GFLOP to ~34 GFLOP.

Sharding: rows of x across 8 cores. Each core:
  1. loads its [N/8, D] row block, computes row norms + normalized rows
  2. computes partial G_c = xn_c.T @ x_c  (f32r matmuls, f32 PSUM accum)
  3. AllReduce(G) across the 8 cores
  4. out_c = sigmoid(xn_c @ G)
The host concatenates the 8 row blocks.

Matmuls run in float32r (full PE rate, ~1.5e-4 rel accuracy vs ~2.4e-3
for bf16). xn.T for step 4 is built with PE transposes (hidden under the
AllReduce window).
"""

import numpy as np

import concourse.bass as bass  # noqa: F401  (engine types come via nc)
import concourse.mybir as mybir
import concourse.tile as tile
from concourse import bacc
from concourse.bass_utils import run_bass_kernel_spmd
from concourse.masks import make_identity

F32 = mybir.dt.float32
F32R = mybir.dt.float32r
BF16 = mybir.dt.bfloat16
AFT = mybir.ActivationFunctionType

N, D = 8192, 1024
NCORES = 8
R = N // NCORES  # rows per core
P = 128
RT = R // P      # row tiles per core
KT = D // P      # contraction tiles (mm2) / G row tiles
FD = 512         # matmul moving free dim (one PSUM bank of f32)
NH = D // FD


def _emit_body(tc, xb, out, mm_dt, ar_dt, ctx):
    nc = tc.nc
    xb_t = xb.rearrange("(rt p) d -> rt p d", p=P)
    out_t = out.rearrange("(rt p) d -> rt p d", p=P)
    pe_transpose_f32 = mm_dt != BF16

    persist = ctx.enter_context(tc.tile_pool(name="persist", bufs=1))
    load = ctx.enter_context(tc.tile_pool(name="load", bufs=3))
    small = ctx.enter_context(tc.tile_pool(name="small", bufs=1))
    gloc = ctx.enter_context(tc.tile_pool(name="gloc", bufs=3))
    gstage = ctx.enter_context(tc.tile_pool(name="gstage", bufs=3))
    ostage = ctx.enter_context(tc.tile_pool(name="ostage", bufs=4))
    ps = ctx.enter_context(tc.tile_pool(name="ps", bufs=5, space="PSUM"))
    tp = ctx.enter_context(tc.tile_pool(name="tp", bufs=2, space="PSUM"))
    dram = ctx.enter_context(tc.tile_pool(name="dram", bufs=1, space="DRAM"))

    ident_dt = F32 if pe_transpose_f32 else BF16
    ident = persist.tile([P, P], ident_dt, tag="ident")
    make_identity(nc, ident)

    # ---- phase 0: load row block, norms, casts ----
    xbr, xnr = [], []
    for rt in range(RT):
        xf = load.tile([P, D], F32, tag="xf")
        nc.sync.dma_start(out=xf, in_=xb_t[rt])
        t_xbr = persist.tile([P, D], mm_dt, tag=f"xbr{rt}")
        nc.vector.tensor_copy(out=t_xbr, in_=xf)
        sq = load.tile([P, D], BF16, tag="sq")
        ss = small.tile([P, 1], F32, tag=f"ss{rt}")
        nc.scalar.activation(out=sq, in_=xf, func=AFT.Square, accum_out=ss)
        nrm = small.tile([P, 1], F32, tag=f"nrm{rt}")
        nc.scalar.sqrt(nrm, ss)
        rn = small.tile([P, 1], F32, tag=f"rn{rt}")
        nc.vector.reciprocal(rn, nrm)
        t_xnr = persist.tile([P, D], mm_dt, tag=f"xnr{rt}")
        nc.vector.tensor_scalar_mul(t_xnr, xf, rn)
        xbr.append(t_xbr)
        xnr.append(t_xnr)

    # ---- phase 1: G_c = xn_c.T @ x_c  -> DRAM bounce for AllReduce ----
    g_in = dram.tile([D, D], ar_dt, tag="g_in")
    g_out = dram.tile([D, D], ar_dt, tag="g_out")
    g_in_t = g_in.rearrange("(kt p) d -> kt p d", p=P)
    g_out_t = g_out.rearrange("(kt p) d -> kt p d", p=P)
    for mt in range(KT):
        gl = gloc.tile([P, D], ar_dt, tag="gloc")
        for nh in range(NH):
            ps_g = ps.tile([P, FD], F32, tag="acc")
            for rt in range(RT):
                nc.tensor.matmul(
                    ps_g,
                    lhsT=xnr[rt][:, mt * P:(mt + 1) * P],
                    rhs=xbr[rt][:, nh * FD:(nh + 1) * FD],
                    start=(rt == 0),
                    stop=(rt == RT - 1),
                )
            nc.vector.tensor_copy(out=gl[:, nh * FD:(nh + 1) * FD], in_=ps_g)
        nc.sync.dma_start(out=g_in_t[mt], in_=gl)

    # ---- phase 1.5: xnT (overlaps mm1 / AllReduce) ----
    xnT = []
    for kt in range(KT):
        t_xnT = persist.tile([P, D], mm_dt, tag=f"xnT{kt}")
        for rt in range(RT):
            src = xnr[rt][:, kt * P:(kt + 1) * P]
            if pe_transpose_f32:
                tpt = tp.tile([P, P], F32, tag="tp")
                nc.tensor.transpose(tpt, src.bitcast(F32), ident)
                nc.vector.tensor_copy(out=t_xnT[:, rt * P:(rt + 1) * P], in_=tpt)
            else:
                nc.sync.dma_start_transpose(
                    out=t_xnT[:, rt * P:(rt + 1) * P], in_=src
                )
        xnT.append(t_xnT)

    # ---- phase 2: AllReduce G ----
    nc.gpsimd.collective_compute(
        "AllReduce",
        mybir.AluOpType.add,
        replica_groups=[list(range(NCORES))],
        ins=[g_in.opt()],
        outs=[g_out.opt()],
    )

    # ---- phase 3: load G back (+ round to f32r) ----
    gr = []
    for kt in range(KT):
        t_gr = persist.tile([P, D], mm_dt, tag=f"gr{kt}")
        if mm_dt == F32R:
            gs = gstage.tile([P, D], F32, tag="gs")
            nc.sync.dma_start(out=gs, in_=g_out_t[kt])
            nc.vector.tensor_copy(out=t_gr, in_=gs)
        else:
            nc.sync.dma_start(out=t_gr, in_=g_out_t[kt])
        gr.append(t_gr)

    # ---- phase 4: out_c = sigmoid(xn_c @ G) ----
    for mt in range(RT):
        for nh in range(NH):
            ps_z = ps.tile([P, FD], F32, tag="acc")
            for kt in range(KT):
                nc.tensor.matmul(
                    ps_z,
                    lhsT=xnT[kt][:, mt * P:(mt + 1) * P],
                    rhs=gr[kt][:, nh * FD:(nh + 1) * FD],
                    start=(kt == 0),
                    stop=(kt == KT - 1),
                )
            ob = ostage.tile([P, FD], F32, tag="ob")
            nc.scalar.activation(out=ob, in_=ps_z, func=AFT.Sigmoid)
            nc.sync.dma_start(out=out_t[mt][:, nh * FD:(nh + 1) * FD], in_=ob)


def build(mm_dt=F32R, ar_dt=F32):
    from contextlib import ExitStack

    nc = bacc.Bacc("TRN2", target_bir_lowering=False, debug=False,
                   num_devices=NCORES)
    xb = nc.dram_tensor("xb", [R, D], F32, kind="ExternalInput").ap()
    out = nc.dram_tensor("out", [R, D], F32, kind="ExternalOutput").ap()
    with tile.TileContext(nc) as tc:
        with ExitStack() as ctx:
            _emit_body(tc, xb, out, mm_dt, ar_dt, ctx)
    nc.compile()
    return nc


_NC_CACHE = {}


def _get_nc(mm_dt=F32R, ar_dt=F32):
    key = (str(mm_dt), str(ar_dt))
    if key not in _NC_CACHE:
        _NC_CACHE[key] = build(mm_dt, ar_dt)
    return _NC_CACHE[key]


def kernel(x: np.ndarray) -> np.ndarray:
    x = np.asarray(x, dtype=np.float32)
    assert x.shape == (N, D), x.shape
    nc = _get_nc()
    in_maps = [{"xb": x[c * R:(c + 1) * R]} for c in range(NCORES)]
    res = run_bass_kernel_spmd(nc, in_maps, list(range(NCORES)))
    return np.concatenate([res.results[c]["out"] for c in range(NCORES)], axis=0)


# revision 5
# speedup vs baseline: 1.0489x; 1.0489x over previous
"""Trainium2 Bass kernel for CosineAttention:

    out = sigmoid((xn @ xn.T) @ x)   where xn = x / ||x_row||

Key algebraic optimization: reassociate (xn @ xn.T) @ x = xn @ (xn.T @ x).
G = xn.T @ x is [D, D] — the O(N^2 D) similarity matrix is never formed.
Total work drops from ~275 GFLOP to ~34 GFLOP.

Sharding: rows of x across 8 cores. Each core:
  1. loads its [N/8, D] row block, computes row norms + normalized rows
  2. computes partial G_c = xn_c.T @ x_c  (f32r matmuls, f32 PSUM accum)
  3. AllReduce(G) across the 8 cores (2 column chunks, overlapped)
  4. out_c = sigmoid(xn_c @ G)
The host concatenates the 8 row blocks.

Schedule details:
  - a tiny warmup AllReduce at kernel start absorbs the first-collective
    barrier + setup latency
  - mm1 is row-tile-outer so PE starts as soon as tile 0 is loaded
  - the G AllReduce is split into two column halves; half 0 reduces while
    mm1 computes half 1, and mm2 on half 0 overlaps the half-1 reduce
  - xn.T is built with PE transposes after mm1 (f32 exact)
"""

import numpy as np

import concourse.bass as bass  # noqa: F401
import concourse.mybir as mybir
import concourse.tile as tile
from concourse import bacc
from concourse.bass_utils import run_bass_kernel_spmd
from concourse.masks import make_identity

F32 = mybir.dt.float32
F32R = mybir.dt.float32r
BF16 = mybir.dt.bfloat16
AFT = mybir.ActivationFunctionType

N, D = 8192, 1024
NCORES = 8
R = N // NCORES  # rows per core
P = 128
RT = R // P      # row tiles per core
KT = D // P      # contraction tiles (mm2) / G row tiles
FD = 512         # matmul moving free dim (one PSUM bank of f32)
NH = D // FD     # column halves
GROUPS = [list(range(NCORES))]


def _emit_body(tc, xb, out, mm_dt, ar_dt, ctx):
    nc = tc.nc
    xb_t = xb.rearrange("(rt p) d -> rt p d", p=P)
    out_t = out.rearrange("(rt p) d -> rt p d", p=P)
    f32r_mode = mm_dt == F32R

    persist = ctx.enter_context(tc.tile_pool(name="persist", bufs=1))
    load = ctx.enter_context(tc.tile_pool(name="load", bufs=3))
    small = ctx.enter_context(tc.tile_pool(name="small", bufs=1))
    gloc = ctx.enter_context(tc.tile_pool(name="gloc", bufs=3))
    gstage = ctx.enter_context(tc.tile_pool(name="gstage", bufs=3))
    ostage = ctx.enter_context(tc.tile_pool(name="ostage", bufs=3))
    ps = ctx.enter_context(tc.tile_pool(name="ps", bufs=1, space="PSUM"))
    dram = ctx.enter_context(tc.tile_pool(name="dram", bufs=1, space="DRAM"))

    # ---- warmup collective: absorbs first-collective barrier/setup ----
    w_sb = small.tile([P, 4], F32, tag="w_sb")
    nc.vector.memset(w_sb, 0.0)
    w_in = dram.tile([P, 4], F32, tag="w_in")
    w_out = dram.tile([P, 4], F32, tag="w_out")
    nc.sync.dma_start(out=w_in, in_=w_sb)
    nc.gpsimd.collective_compute(
        "AllReduce", mybir.AluOpType.add, replica_groups=GROUPS,
        ins=[w_in.opt()], outs=[w_out.opt()],
    )

    ident = persist.tile([P, P], F32, tag="ident")
    make_identity(nc, ident)

    # ---- phase 0: load row block, norms, casts ----
    xbr, xnr = [], []
    for rt in range(RT):
        xf = load.tile([P, D], F32, tag="xf")
        nc.sync.dma_start(out=xf, in_=xb_t[rt])
        t_xbr = persist.tile([P, D], mm_dt, tag=f"xbr{rt}")
        nc.vector.tensor_copy(out=t_xbr, in_=xf)
        sq = load.tile([P, D], BF16, tag="sq")
        ss = small.tile([P, 1], F32, tag=f"ss{rt}")
        nc.scalar.activation(out=sq, in_=xf, func=AFT.Square, accum_out=ss)
        nrm = small.tile([P, 1], F32, tag=f"nrm{rt}")
        nc.scalar.sqrt(nrm, ss)
        rn = small.tile([P, 1], F32, tag=f"rn{rt}")
        nc.vector.reciprocal(rn, nrm)
        t_xnr = persist.tile([P, D], mm_dt, tag=f"xnr{rt}")
        nc.vector.tensor_scalar_mul(t_xnr, xf, rn)
        xbr.append(t_xbr)
        xnr.append(t_xnr)

    # ---- phase 1: G_c = xn_c.T @ x_c, wave per column half ----
    # rt-outer so PE starts on tile 0 while later tiles stream in.
    g_in, g_out = [], []
    for nh in range(NH):
        g_in.append(dram.tile([D, FD], ar_dt, tag=f"g_in{nh}", name=f"g_in{nh}"))
        g_out.append(dram.tile([D, FD], ar_dt, tag=f"g_out{nh}", name=f"g_out{nh}"))
    for nh in range(NH):
        psg = [ps.tile([P, FD], F32, tag=f"acc{mt}", name=f"psg{nh}_{mt}") for mt in range(KT)]
        for rt in range(RT):
            for mt in range(KT):
                nc.tensor.matmul(
                    psg[mt],
                    lhsT=xnr[rt][:, mt * P:(mt + 1) * P],
                    rhs=xbr[rt][:, nh * FD:(nh + 1) * FD],
                    start=(rt == 0),
                    stop=(rt == RT - 1),
                )
        g_in_t = g_in[nh].rearrange("(mt p) f -> mt p f", p=P)
        for mt in range(KT):
            gl = gloc.tile([P, FD], ar_dt, tag="gloc")
            nc.vector.tensor_copy(out=gl, in_=psg[mt])
            nc.sync.dma_start(out=g_in_t[mt], in_=gl)
        # chunked AllReduce: half nh reduces while the other half computes
        nc.gpsimd.collective_compute(
            "AllReduce", mybir.AluOpType.add, replica_groups=GROUPS,
            ins=[g_in[nh].opt()], outs=[g_out[nh].opt()],
        )

    # ---- phase 1.5: xnT via PE transposes (after mm1 on PE) ----
    xnT = []
    for kt in range(KT):
        t_xnT = persist.tile([P, D], mm_dt, tag=f"xnT{kt}")
        for rt in range(RT):
            src = xnr[rt][:, kt * P:(kt + 1) * P]
            if f32r_mode:
                tpt = ps.tile([P, P], F32, tag=f"acc{rt % 2}", name=f"tp{kt}_{rt}")
                nc.tensor.transpose(tpt, src.bitcast(F32), ident)
                nc.vector.tensor_copy(out=t_xnT[:, rt * P:(rt + 1) * P], in_=tpt)
            else:
                nc.sync.dma_start_transpose(
                    out=t_xnT[:, rt * P:(rt + 1) * P], in_=src
                )
        xnT.append(t_xnT)

    # ---- phases 3+4 per column half: load G half, round, mm2, sigmoid ----
    for nh in range(NH):
        g_out_t = g_out[nh].rearrange("(kt p) f -> kt p f", p=P)
        gr = []
        for kt in range(KT):
            t_gr = persist.tile([P, FD], mm_dt, tag=f"gr{nh}_{kt}")
            if f32r_mode:
                gs = gstage.tile([P, FD], F32, tag="gs")
                nc.sync.dma_start(out=gs, in_=g_out_t[kt])
                nc.vector.tensor_copy(out=t_gr, in_=gs)
            else:
                nc.sync.dma_start(out=t_gr, in_=g_out_t[kt])
            gr.append(t_gr)
        for mt in range(RT):
            ps_z = ps.tile([P, FD], F32, tag=f"acc{mt}")
            for kt in range(KT):
                nc.tensor.matmul(
                    ps_z,
                    lhsT=xnT[kt][:, mt * P:(mt + 1) * P],
                    rhs=gr[kt],
                    start=(kt == 0),
                    stop=(kt == KT - 1),
                )
            ob = ostage.tile([P, FD], F32, tag="ob")
            nc.scalar.activation(out=ob, in_=ps_z, func=AFT.Sigmoid)
            nc.sync.dma_start(out=out_t[mt][:, nh * FD:(nh + 1) * FD], in_=ob)


def build(mm_dt=F32R, ar_dt=F32):
    from contextlib import ExitStack

    nc = bacc.Bacc("TRN2", target_bir_lowering=False, debug=False,
                   num_devices=NCORES)
    xb = nc.dram_tensor("xb", [R, D], F32, kind="ExternalInput").ap()
    out = nc.dram_tensor("out", [R, D], F32, kind="ExternalOutput").ap()
    with tile.TileContext(nc) as tc:
        with ExitStack() as ctx:
            _emit_body(tc, xb, out, mm_dt, ar_dt, ctx)
    nc.compile()
    return nc


_NC_CACHE = {}


def _get_nc(mm_dt=F32R, ar_dt=F32):
    key = (str(mm_dt), str(ar_dt))
    if key not in _NC_CACHE:
        _NC_CACHE[key] = build(mm_dt, ar_dt)
    return _NC_CACHE[key]


def kernel(x: np.ndarray) -> np.ndarray:
    x = np.asarray(x, dtype=np.float32)
    assert x.shape == (N, D), x.shape
    nc = _get_nc()
    in_maps = [{"xb": x[c * R:(c + 1) * R]} for c in range(NCORES)]
    res = run_bass_kernel_spmd(nc, in_maps, list(range(NCORES)))
    return np.concatenate([res.results[c]["out"] for c in range(NCORES)], axis=0)
